# revision 6
# baseline (speedup 1.0000x reference)
"""Multi-head attention (B=8, N=1024, C=768, H=12) on 8 TRN2 NeuronCores.

Data-parallel over batch: core b computes batch element b end-to-end.

Per-core dataflow (all matmuls in float32r — 1 cycle/row on the PE with
~1.5e-4 relative rounding error; PSUM accumulation is fp32):

  qkT[f,t]  = wqkT.T @ xT          (q,k kept feature-major for the S matmul)
  v[t,f]    = xT.T @ wvT           (token-major, padded with a ones column
                                    per head so the PV matmul also produces
                                    the softmax denominator)
  ST[kj,qi] = kT.T @ qT            (per head, K=64)
  E         = exp(SCALE * ST)      (ScalarE, no max-subtraction: |S|<9)
  oT'[d+1,qi] = [v_h|1].T @ E      (accumulated over kj chunks; row 64 holds
                                    the denominator colsum)
  oT_h      = oT'[0:64] * bcast(1/colsum)   (rank-1 PE broadcast + DVE mul)
  outT[f,t] = woT.T @ oT + b_out   (bias via per-partition tensor_scalar add)

Host side transposes x/weights into the layouts above and transposes the
outT result back.
"""

import os

import numpy as np

import concourse.bass as bass
import concourse.tile as tile
from concourse import mybir
from concourse.bass_utils import run_bass_kernel_spmd

B, N, C = 8, 1024, 768
H, D = 12, 64
SCALE = D ** -0.5
CC = C // 128          # 6 contraction chunks
QKF = (2 * C) // 128   # 12 q|k feature chunks
NT2 = N // 512         # 2 free-dim chunks of 512
NT8 = N // 128         # 8 partition chunks of 128
F32 = mybir.dt.float32
F32R = mybir.dt.float32r

_N_CORES = 8


def _split_multiwaits(nc, max_waits: int = 1):
    """The pinned walrus codegen supports one embedded sync-wait per engine
    instruction (single EVENTS slot in the TPB ISA).  Tile's tail drain /
    barriers accumulate several; hoist all-but-one wait onto same-engine
    NoOps placed immediately before the instruction (waits AND, so order is
    irrelevant)."""
    n_split = 0
    for f in nc.m.functions:
        for blk in f.blocks:
            insts = blk.instructions
            if not any(
                ins.sync_info is not None and len(ins.sync_info.on_wait) > max_waits
                for ins in insts
            ):
                continue
            new_list = []
            for ins in insts:
                si = ins.sync_info
                if si is not None and len(si.on_wait) > max_waits:
                    waits = list(si.on_wait)
                    hoist, keep = waits[:-max_waits], waits[-max_waits:]
                    for w in hoist:
                        nop = mybir.InstNoOp(name=nc.get_next_instruction_name())
                        nop.engine = ins.engine
                        nop.sync_info = mybir.SyncInfo(on_wait=[w], on_update=[])
                        new_list.append(nop)
                        n_split += 1
                    ins.sync_info = mybir.SyncInfo(
                        on_wait=keep, on_update=list(si.on_update)
                    )
                new_list.append(ins)
            blk.instructions = new_list
    return n_split


def _build(split: bool = True):
    nc = bass.Bass()
    xT = nc.declare_dram_parameter("xT", [C, N], F32R, isOutput=False)
    wqkT = nc.declare_dram_parameter("wqkT", [C, 2 * C], F32R, isOutput=False)
    wvT = nc.declare_dram_parameter("wvT", [C, C], F32R, isOutput=False)
    woT = nc.declare_dram_parameter("woT", [C, C], F32R, isOutput=False)
    bo = nc.declare_dram_parameter("bo", [C, 1], F32, isOutput=False)
    outT = nc.declare_dram_parameter("outT", [C, N], F32, isOutput=True)

    with tile.TileContext(nc) as tc:
        with (
            tc.tile_pool(name="persist", bufs=1) as persist,
            tc.tile_pool(name="psum", bufs=1, space="PSUM") as psum,
        ):
            qkT = [persist.tile([128, N], F32R, tag=f"qkT{j}", name=f"qkT{j}") for j in range(QKF)]
            v_sb = [
                persist.tile([128, H * (D + 1)], F32R, tag=f"v{t}", name=f"v{t}") for t in range(NT8)
            ]
            oT = [persist.tile([128, N], F32R, tag=f"oT{c}", name=f"oT{c}") for c in range(CC)]
            bo_t = [persist.tile([128, 1], F32, tag=f"bo{c}", name=f"bo{c}") for c in range(CC)]
            for c in range(CC):
                nc.sync.dma_start(out=bo_t[c], in_=bo[c * 128 : (c + 1) * 128, :])

            # constants (fp32 staging -> f32r so the verifier sees a rounding op)
            ones12_f = persist.tile([128, H], F32, tag="ones12f")
            nc.vector.memset(ones12_f, 1.0)
            ones1_f = persist.tile([1, D], F32, tag="ones1f")
            nc.vector.memset(ones1_f, 1.0)
            ones1_r = persist.tile([1, D], F32R, tag="ones1r")
            nc.vector.tensor_copy(ones1_r, ones1_f)

            # ---------------- phase 1: projections ----------------
            with tc.tile_pool(name="ph1", bufs=1) as ph1:
                xr = [ph1.tile([128, N], F32R, tag=f"xr{c}", name=f"xr{c}") for c in range(CC)]
                wqk = [ph1.tile([128, 2 * C], F32R, tag=f"wqk{c}", name=f"wqk{c}") for c in range(CC)]
                wv = [ph1.tile([128, C], F32R, tag=f"wv{c}", name=f"wv{c}") for c in range(CC)]
                for c in range(CC):
                    sl = slice(c * 128, (c + 1) * 128)
                    nc.sync.dma_start(out=xr[c], in_=xT[sl, :])
                    nc.sync.dma_start(out=wqk[c], in_=wqkT[sl, :])
                    nc.sync.dma_start(out=wv[c], in_=wvT[sl, :])

                # 1a: qkT[j][:, qs] = sum_c wqk[c][:,j-block].T @ xr[c][:, qs]
                for j in range(QKF):
                    for t2 in range(NT2):
                        p = psum.tile([128, 512], F32, tag="mm", name="mm", bufs=4)
                        for c in range(CC):
                            nc.tensor.matmul(
                                p,
                                wqk[c][:, j * 128 : (j + 1) * 128],
                                xr[c][:, t2 * 512 : (t2 + 1) * 512],
                                start=(c == 0),
                                stop=(c == CC - 1),
                            )
                        nc.vector.tensor_copy(
                            qkT[j][:, t2 * 512 : (t2 + 1) * 512], p
                        )

                # 1b: v[t][:, h*65:h*65+64] = sum_c xr[c][:,t-block].T @ wv[c]
                for t8 in range(NT8):
                    for nh in range(2):
                        p = psum.tile([128, 384], F32, tag="mm", name="mm", bufs=4)
                        for c in range(CC):
                            nc.tensor.matmul(
                                p,
                                xr[c][:, t8 * 128 : (t8 + 1) * 128],
                                wv[c][:, nh * 384 : (nh + 1) * 384],
                                start=(c == 0),
                                stop=(c == CC - 1),
                            )
                        v_view = v_sb[t8].rearrange("p (h e) -> p h e", e=D + 1)
                        nc.vector.tensor_copy(
                            v_view[:, nh * 6 : (nh + 1) * 6, 0:D],
                            p.rearrange("p (h d) -> p h d", d=D),
                        )
                    # ones column per head (for the colsum row of PV)
                    v_view = v_sb[t8].rearrange("p (h e) -> p h e", e=D + 1)
                    nc.vector.tensor_copy(
                        v_view[:, :, D : D + 1], ones12_f.unsqueeze(2)
                    )

            # ---------------- phase 2: attention ----------------
            with tc.tile_pool(name="ph2", bufs=10) as ph2, tc.tile_pool(
                name="ph2s", bufs=3
            ) as ph2s:
                for h in range(H):
                    kt = qkT[CC + h // 2]
                    qt = qkT[h // 2]
                    po = (h % 2) * 64  # partition offset of this head
                    for t2 in range(NT2):
                        qs = slice(t2 * 512, (t2 + 1) * 512)
                        ex = []
                        for kc in range(NT8):
                            st = psum.tile([128, 512], F32, tag="mm", name="mm", bufs=4)
                            nc.tensor.matmul(
                                st,
                                kt[po : po + D, kc * 128 : (kc + 1) * 128],
                                qt[po : po + D, qs],
                                start=True,
                                stop=True,
                            )
                            e = ph2.tile([128, 512], F32R, tag="exps")
                            nc.scalar.activation(
                                e, st, mybir.ActivationFunctionType.Exp, scale=SCALE
                            )
                            ex.append(e)
                        o = psum.tile([D + 1, 512], F32, tag="ops", name="ops", bufs=2)
                        for kc in range(NT8):
                            nc.tensor.matmul(
                                o,
                                v_sb[kc][:, h * (D + 1) : (h + 1) * (D + 1)],
                                ex[kc],
                                start=(kc == 0),
                                stop=(kc == NT8 - 1),
                            )
                        rec = ph2s.tile([1, 512], F32, tag="rec")
                        nc.vector.reciprocal(rec, o[D : D + 1, :])
                        rec_r = ph2s.tile([1, 512], F32R, tag="recr")
                        nc.vector.tensor_copy(rec_r, rec)
                        bc = psum.tile([D, 512], F32, tag="bcps", name="bcps", bufs=2)
                        nc.tensor.matmul(bc, ones1_r, rec_r, start=True, stop=True)
                        bc_sb = ph2s.tile([D, 512], F32, tag="bcsb")
                        nc.vector.tensor_copy(bc_sb, bc)
                        nc.vector.tensor_mul(
                            oT[h // 2][po : po + D, qs], o[0:D, :], bc_sb
                        )

            # ---------------- phase 3: output projection ----------------
            with tc.tile_pool(name="ph3", bufs=1) as ph3, tc.tile_pool(
                name="ph3o", bufs=3
            ) as ph3o:
                wo = [ph3.tile([128, C], F32R, tag=f"wo{c}", name=f"wo{c}") for c in range(CC)]
                for c in range(CC):
                    nc.sync.dma_start(
                        out=wo[c], in_=woT[c * 128 : (c + 1) * 128, :]
                    )
                for fc in range(CC):
                    for t2 in range(NT2):
                        p = psum.tile([128, 512], F32, tag="mm", name="mm", bufs=4)
                        for c in range(CC):
                            nc.tensor.matmul(
                                p,
                                wo[c][:, fc * 128 : (fc + 1) * 128],
                                oT[c][:, t2 * 512 : (t2 + 1) * 512],
                                start=(c == 0),
                                stop=(c == CC - 1),
                            )
                        ot = ph3o.tile([128, 512], F32, tag="outsb")
                        nc.vector.tensor_scalar_add(ot, p, bo_t[fc])
                        nc.sync.dma_start(
                            out=outT[
                                fc * 128 : (fc + 1) * 128,
                                t2 * 512 : (t2 + 1) * 512,
                            ],
                            in_=ot,
                        )

    if split:
        _split_multiwaits(nc)
    return nc


_NC = None


def _get_nc():
    global _NC
    if _NC is None:
        _NC = _build()
    return _NC


def kernel(x, w_qkv, w_out, b_out):
    x = np.asarray(x, dtype=np.float32)
    w_qkv = np.asarray(w_qkv, dtype=np.float32)
    w_out = np.asarray(w_out, dtype=np.float32)
    b_out = np.asarray(b_out, dtype=np.float32)

    wqkT = np.ascontiguousarray(w_qkv[: 2 * C].T)
    wvT = np.ascontiguousarray(w_qkv[2 * C :].T)
    woT = np.ascontiguousarray(w_out.T)
    bo = np.ascontiguousarray(b_out.reshape(C, 1))

    in_maps = [
        {
            "xT": np.ascontiguousarray(x[b].T),
            "wqkT": wqkT,
            "wvT": wvT,
            "woT": woT,
            "bo": bo,
        }
        for b in range(B)
    ]

    nc = _get_nc()
    trace = bool(os.environ.get("KERNEL_TRACE"))
    res = run_bass_kernel_spmd(
        nc, in_maps, list(range(_N_CORES)), trace=trace
    )
    if trace:
        print(f"HW exec time: {res.exec_time_ns} ns")
        if res.instructions_and_trace is not None:
            print(f"trace: {res.instructions_and_trace[1]}")

    out = np.empty((B, N, C), dtype=np.float32)
    for b in range(B):
        out[b] = res.results[b]["outT"].T
    return out


# revision 8
# speedup vs baseline: 1.1611x; 1.1611x over previous
"""Multi-head attention (B=8, N=1024, C=768, H=12) on 8 TRN2 NeuronCores.

Data-parallel over batch: core b computes batch element b end-to-end.

Per-core dataflow (matmul operands in fp16 — 1 cycle/row on the PE, fp32
PSUM accumulation; measured end-to-end relative error ~7e-4):

  qkT[f,t]  = wqkT.T @ xT          (q,k kept feature-major for the S matmul)
  v[t,f]    = xT.T @ wvT           (token-major, padded with a ones column
                                    per head so the PV matmul also produces
                                    the softmax denominator)
  ST[kj,qi] = kT.T @ qT            (per head, K=64)
  E         = exp(SCALE * ST)      (ScalarE, no max-subtraction: |S|<9 so
                                    exp stays well inside fp16 range)
  oT'[d+1,qi] = [v_h|1].T @ E      (accumulated over kj chunks; row 64 holds
                                    the denominator colsum)
  oT_h      = oT'[0:64] * bcast(1/colsum)   (rank-1 PE broadcast + DVE mul)
  outT[f,t] = woT.T @ oT + b_out   (bias via per-partition tensor_scalar add,
                                    fp32 all the way to the output)

Host side casts x/weights to fp16 in the layouts above and transposes the
fp32 outT result back.
"""

import os

import numpy as np

import concourse.bass as bass
import concourse.tile as tile
from concourse import mybir
from concourse.bass_utils import run_bass_kernel_spmd

B, N, C = 8, 1024, 768
H, D = 12, 64
SCALE = D ** -0.5
CC = C // 128          # 6 contraction chunks
QKF = (2 * C) // 128   # 12 q|k feature chunks
NT2 = N // 512         # 2 free-dim chunks of 512
NT8 = N // 128         # 8 partition chunks of 128
F32 = mybir.dt.float32
F16 = mybir.dt.float16

_N_CORES = 8


def _split_multiwaits(nc, max_waits: int = 1):
    """The pinned walrus codegen supports one embedded sync-wait per engine
    instruction (single EVENTS slot in the TPB ISA).  Tile's tail drain /
    barriers accumulate several; hoist all-but-one wait onto same-engine
    NoOps placed immediately before the instruction (waits AND, so order is
    irrelevant)."""
    n_split = 0
    for f in nc.m.functions:
        for blk in f.blocks:
            insts = blk.instructions
            if not any(
                ins.sync_info is not None and len(ins.sync_info.on_wait) > max_waits
                for ins in insts
            ):
                continue
            new_list = []
            for ins in insts:
                si = ins.sync_info
                if si is not None and len(si.on_wait) > max_waits:
                    waits = list(si.on_wait)
                    hoist, keep = waits[:-max_waits], waits[-max_waits:]
                    for w in hoist:
                        nop = mybir.InstNoOp(name=nc.get_next_instruction_name())
                        nop.engine = ins.engine
                        nop.sync_info = mybir.SyncInfo(on_wait=[w], on_update=[])
                        new_list.append(nop)
                        n_split += 1
                    ins.sync_info = mybir.SyncInfo(
                        on_wait=keep, on_update=list(si.on_update)
                    )
                new_list.append(ins)
            blk.instructions = new_list
    return n_split


def _build(split: bool = True):
    nc = bass.Bass()
    xT = nc.declare_dram_parameter("xT", [C, N], F16, isOutput=False)
    wqkT = nc.declare_dram_parameter("wqkT", [C, 2 * C], F16, isOutput=False)
    wvT = nc.declare_dram_parameter("wvT", [C, C], F16, isOutput=False)
    woT = nc.declare_dram_parameter("woT", [C, C], F16, isOutput=False)
    bo = nc.declare_dram_parameter("bo", [C, 1], F32, isOutput=False)
    outT = nc.declare_dram_parameter("outT", [C, N], F32, isOutput=True)

    with tile.TileContext(nc) as tc:
        with (
            tc.tile_pool(name="sb", bufs=1) as sb,
            tc.tile_pool(name="psum", bufs=1, space="PSUM") as psum,
        ):
            qkT = [
                sb.tile([128, N], F16, tag=f"qkT{j}", name=f"qkT{j}")
                for j in range(QKF)
            ]
            v_sb = [
                sb.tile([128, H * (D + 1)], F16, tag=f"v{t}", name=f"v{t}")
                for t in range(NT8)
            ]
            oT = [sb.tile([128, N], F16, tag=f"oT{c}", name=f"oT{c}") for c in range(CC)]
            bo_t = [sb.tile([128, 1], F32, tag=f"bo{c}", name=f"bo{c}") for c in range(CC)]
            xr = [sb.tile([128, N], F16, tag=f"xr{c}", name=f"xr{c}") for c in range(CC)]
            wqk = [
                sb.tile([128, 2 * C], F16, tag=f"wqk{c}", name=f"wqk{c}")
                for c in range(CC)
            ]
            wv = [sb.tile([128, C], F16, tag=f"wv{c}", name=f"wv{c}") for c in range(CC)]
            wo = [sb.tile([128, C], F16, tag=f"wo{c}", name=f"wo{c}") for c in range(CC)]

            for c in range(CC):
                sl = slice(c * 128, (c + 1) * 128)
                nc.sync.dma_start(out=xr[c], in_=xT[sl, :])
                nc.sync.dma_start(out=wqk[c], in_=wqkT[sl, :])
                nc.sync.dma_start(out=wv[c], in_=wvT[sl, :])
                nc.sync.dma_start(out=wo[c], in_=woT[sl, :])
                nc.sync.dma_start(out=bo_t[c], in_=bo[sl, :])

            ones12 = sb.tile([128, H], F16, tag="ones12")
            nc.vector.memset(ones12, 1.0)
            ones1h = sb.tile([1, D], F16, tag="ones1h")
            nc.vector.memset(ones1h, 1.0)

            # ---------------- phase 1: projections ----------------
            # 1a: qkT[j][:, qs] = sum_c wqk[c][:, j-block].T @ xr[c][:, qs]
            for j in range(QKF):
                for t2 in range(NT2):
                    p = psum.tile([128, 512], F32, tag="mm", name="mm", bufs=4)
                    for c in range(CC):
                        nc.tensor.matmul(
                            p,
                            wqk[c][:, j * 128 : (j + 1) * 128],
                            xr[c][:, t2 * 512 : (t2 + 1) * 512],
                            start=(c == 0),
                            stop=(c == CC - 1),
                        )
                    nc.vector.tensor_copy(qkT[j][:, t2 * 512 : (t2 + 1) * 512], p)

            # 1b: v[t][:, h*65:h*65+64] = sum_c xr[c][:, t-block].T @ wv[c]
            for t8 in range(NT8):
                for nh in range(2):
                    p = psum.tile([128, 384], F32, tag="mm", name="mm", bufs=4)
                    for c in range(CC):
                        nc.tensor.matmul(
                            p,
                            xr[c][:, t8 * 128 : (t8 + 1) * 128],
                            wv[c][:, nh * 384 : (nh + 1) * 384],
                            start=(c == 0),
                            stop=(c == CC - 1),
                        )
                    v_view = v_sb[t8].rearrange("p (h e) -> p h e", e=D + 1)
                    nc.vector.tensor_copy(
                        v_view[:, nh * 6 : (nh + 1) * 6, 0:D],
                        p.rearrange("p (h d) -> p h d", d=D),
                    )
                # ones column per head (for the colsum row of PV)
                v_view = v_sb[t8].rearrange("p (h e) -> p h e", e=D + 1)
                nc.vector.tensor_copy(v_view[:, :, D : D + 1], ones12.unsqueeze(2))

            # ---------------- phase 2: attention ----------------
            with tc.tile_pool(name="ph2", bufs=10) as ph2, tc.tile_pool(
                name="ph2s", bufs=3
            ) as ph2s:
                for h in range(H):
                    kt = qkT[CC + h // 2]
                    qt = qkT[h // 2]
                    po = (h % 2) * 64  # partition offset of this head
                    for t2 in range(NT2):
                        qs = slice(t2 * 512, (t2 + 1) * 512)
                        ex = []
                        for kc in range(NT8):
                            st = psum.tile([128, 512], F32, tag="mm", name="mm", bufs=4)
                            nc.tensor.matmul(
                                st,
                                kt[po : po + D, kc * 128 : (kc + 1) * 128],
                                qt[po : po + D, qs],
                                start=True,
                                stop=True,
                            )
                            e = ph2.tile([128, 512], F16, tag="exps", name="exps")
                            nc.scalar.activation(
                                e, st, mybir.ActivationFunctionType.Exp, scale=SCALE
                            )
                            ex.append(e)
                        o = psum.tile([D + 1, 512], F32, tag="ops", name="ops", bufs=2)
                        for kc in range(NT8):
                            nc.tensor.matmul(
                                o,
                                v_sb[kc][:, h * (D + 1) : (h + 1) * (D + 1)],
                                ex[kc],
                                start=(kc == 0),
                                stop=(kc == NT8 - 1),
                            )
                        rec = ph2s.tile([1, 512], F32, tag="rec", name="rec")
                        nc.vector.reciprocal(rec, o[D : D + 1, :])
                        rec16 = ph2s.tile([1, 512], F16, tag="rec16", name="rec16")
                        nc.vector.tensor_copy(rec16, rec)
                        bc = psum.tile([D, 512], F32, tag="bcps", name="bcps", bufs=2)
                        nc.tensor.matmul(bc, ones1h, rec16, start=True, stop=True)
                        bc_sb = ph2s.tile([D, 512], F32, tag="bcsb", name="bcsb")
                        nc.vector.tensor_copy(bc_sb, bc)
                        nc.vector.tensor_mul(
                            oT[h // 2][po : po + D, qs], o[0:D, :], bc_sb
                        )

            # ---------------- phase 3: output projection ----------------
            with tc.tile_pool(name="ph3o", bufs=3) as ph3o:
                for fc in range(CC):
                    for t2 in range(NT2):
                        p = psum.tile([128, 512], F32, tag="mm", name="mm", bufs=4)
                        for c in range(CC):
                            nc.tensor.matmul(
                                p,
                                wo[c][:, fc * 128 : (fc + 1) * 128],
                                oT[c][:, t2 * 512 : (t2 + 1) * 512],
                                start=(c == 0),
                                stop=(c == CC - 1),
                            )
                        ot = ph3o.tile([128, 512], F32, tag="outsb", name="outsb")
                        nc.vector.tensor_scalar_add(ot, p, bo_t[fc])
                        nc.sync.dma_start(
                            out=outT[
                                fc * 128 : (fc + 1) * 128,
                                t2 * 512 : (t2 + 1) * 512,
                            ],
                            in_=ot,
                        )

    if split:
        _split_multiwaits(nc)
    return nc


_NC = None


def _get_nc():
    global _NC
    if _NC is None:
        _NC = _build()
    return _NC


def kernel(x, w_qkv, w_out, b_out):
    x = np.asarray(x, dtype=np.float32)
    w_qkv = np.asarray(w_qkv, dtype=np.float32)
    w_out = np.asarray(w_out, dtype=np.float32)
    b_out = np.asarray(b_out, dtype=np.float32)

    wqkT = np.ascontiguousarray(w_qkv[: 2 * C].T.astype(np.float16))
    wvT = np.ascontiguousarray(w_qkv[2 * C :].T.astype(np.float16))
    woT = np.ascontiguousarray(w_out.T.astype(np.float16))
    bo = np.ascontiguousarray(b_out.reshape(C, 1))

    in_maps = [
        {
            "xT": np.ascontiguousarray(x[b].T.astype(np.float16)),
            "wqkT": wqkT,
            "wvT": wvT,
            "woT": woT,
            "bo": bo,
        }
        for b in range(B)
    ]

    nc = _get_nc()
    trace = bool(os.environ.get("KERNEL_TRACE"))
    res = run_bass_kernel_spmd(nc, in_maps, list(range(_N_CORES)), trace=trace)
    if trace:
        print(f"HW exec time: {res.exec_time_ns} ns")
        if res.instructions_and_trace is not None:
            print(f"trace: {res.instructions_and_trace[1]}")

    out = np.empty((B, N, C), dtype=np.float32)
    for b in range(B):
        out[b] = res.results[b]["outT"].T
    return out


# revision 12
# speedup vs baseline: 1.2425x; 1.0701x over previous
"""Multi-head attention (B=8, N=1024, C=768, H=12) on 8 TRN2 NeuronCores.

Data-parallel over batch: core b computes batch element b end-to-end.

Per-core dataflow (matmul operands in fp16 — 1 cycle/row on the PE, fp32
PSUM accumulation; measured end-to-end relative error ~7e-4):

  qkT[f,t]  = wqkT.T @ xT          (q,k kept feature-major for the S matmul)
  v[t,f]    = xT.T @ wvT           (token-major, padded with a ones column
                                    per head so the PV matmul also produces
                                    the softmax denominator)
  ST[kj,qi] = kT.T @ qT            (per head, K=64; two 512-wide matmuls
                                    into one 2-bank PSUM tile)
  E         = exp(SCALE * ST)      (ScalarE on [128,1024] tiles — amortizes
                                    the ~352-cycle ACTIVATE overhead; no
                                    max-subtraction: |S|<9, well in fp16)
  oT'[d+1,qi] = [v_h|1].T @ E      (accumulated over kj chunks; row 64 holds
                                    the denominator colsum)
  oT_h      = oT'[0:64] / bcast(colsum)     (rank-1 PE broadcast + DVE div)
  outT[f,t] = woT.T @ oT + b_out   (bias via per-partition tensor_scalar add,
                                    fp32 all the way to the output)

Phase 2 is software-pipelined per head: ST/exp of head h are emitted before
PV/normalize of head h-1, keeping the PE busy while ScalarE works through
the exps (PE gaps > ~3.4us de-warm the HAM clock gate to 1.2 GHz).

Host side casts x/weights to fp16 in the layouts above and transposes the
fp32 outT result back.
"""

import os

import numpy as np

import concourse.bass as bass
import concourse.tile as tile
from concourse import mybir
from concourse.bass_utils import run_bass_kernel_spmd

B, N, C = 8, 1024, 768
H, D = 12, 64
SCALE = D ** -0.5
CC = C // 128          # 6 contraction chunks
QKF = (2 * C) // 128   # 12 q|k feature chunks
NT2 = N // 512         # 2 free-dim chunks of 512
NT8 = N // 128         # 8 partition chunks of 128
F32 = mybir.dt.float32
F16 = mybir.dt.float16

# normalize via DVE tensor_tensor divide (True) or reciprocal+mul (False).
# divide is NOT a valid DVE ALU op on TRN2 (s3s3d3_tt_valid_op) — keep False.
NORM_DIVIDE = False

_N_CORES = 8


def _split_multiwaits(nc, max_waits: int = 1):
    """The pinned walrus codegen supports one embedded sync-wait per engine
    instruction (single EVENTS slot in the TPB ISA).  Tile's tail drain /
    barriers accumulate several; hoist all-but-one wait onto same-engine
    NoOps placed immediately before the instruction (waits AND, so order is
    irrelevant)."""
    n_split = 0
    for f in nc.m.functions:
        for blk in f.blocks:
            insts = blk.instructions
            if not any(
                ins.sync_info is not None and len(ins.sync_info.on_wait) > max_waits
                for ins in insts
            ):
                continue
            new_list = []
            for ins in insts:
                si = ins.sync_info
                if si is not None and len(si.on_wait) > max_waits:
                    waits = list(si.on_wait)
                    hoist, keep = waits[:-max_waits], waits[-max_waits:]
                    for w in hoist:
                        nop = mybir.InstNoOp(name=nc.get_next_instruction_name())
                        nop.engine = ins.engine
                        nop.sync_info = mybir.SyncInfo(on_wait=[w], on_update=[])
                        new_list.append(nop)
                        n_split += 1
                    ins.sync_info = mybir.SyncInfo(
                        on_wait=keep, on_update=list(si.on_update)
                    )
                new_list.append(ins)
            blk.instructions = new_list
    return n_split


def _build(split: bool = True):
    nc = bass.Bass()
    xT = nc.declare_dram_parameter("xT", [C, N], F16, isOutput=False)
    wqkT = nc.declare_dram_parameter("wqkT", [C, 2 * C], F16, isOutput=False)
    wvT = nc.declare_dram_parameter("wvT", [C, C], F16, isOutput=False)
    woT = nc.declare_dram_parameter("woT", [C, C], F16, isOutput=False)
    bo = nc.declare_dram_parameter("bo", [C, 1], F32, isOutput=False)
    outT = nc.declare_dram_parameter("outT", [C, N], F32, isOutput=True)

    with tile.TileContext(nc) as tc:
        with (
            tc.tile_pool(name="sb", bufs=1) as sb,
            tc.tile_pool(name="psum", bufs=1, space="PSUM") as psum,
        ):
            qkT = [
                sb.tile([128, N], F16, tag=f"qkT{j}", name=f"qkT{j}")
                for j in range(QKF)
            ]
            v_sb = [
                sb.tile([128, H * (D + 1)], F16, tag=f"v{t}", name=f"v{t}")
                for t in range(NT8)
            ]
            oT = [sb.tile([128, N], F16, tag=f"oT{c}", name=f"oT{c}") for c in range(CC)]
            bo_t = [sb.tile([128, 1], F32, tag=f"bo{c}", name=f"bo{c}") for c in range(CC)]
            xr = [sb.tile([128, N], F16, tag=f"xr{c}", name=f"xr{c}") for c in range(CC)]
            wqk = [
                sb.tile([128, 2 * C], F16, tag=f"wqk{c}", name=f"wqk{c}")
                for c in range(CC)
            ]
            wv = [sb.tile([128, C], F16, tag=f"wv{c}", name=f"wv{c}") for c in range(CC)]
            wo = [sb.tile([128, C], F16, tag=f"wo{c}", name=f"wo{c}") for c in range(CC)]

            for c in range(CC):
                sl = slice(c * 128, (c + 1) * 128)
                nc.sync.dma_start(out=xr[c], in_=xT[sl, :])
                nc.sync.dma_start(out=wqk[c], in_=wqkT[sl, :])
                nc.sync.dma_start(out=wv[c], in_=wvT[sl, :])
                nc.sync.dma_start(out=wo[c], in_=woT[sl, :])
                nc.sync.dma_start(out=bo_t[c], in_=bo[sl, :])

            ones12 = sb.tile([128, H], F16, tag="ones12")
            nc.vector.memset(ones12, 1.0)
            ones1h = sb.tile([1, D], F16, tag="ones1h")
            nc.vector.memset(ones1h, 1.0)

            # ---------------- phase 1: projections ----------------
            # 1a: qkT[j] = sum_c wqk[c][:, j-block].T @ xr[c]   (both 512-halves
            # of the token dim accumulate into one 2-bank PSUM tile)
            for j in range(QKF):
                p = psum.tile([128, N], F32, tag="big", name="big", bufs=2)
                for c in range(CC):
                    for t2 in range(NT2):
                        nc.tensor.matmul(
                            p[:, t2 * 512 : (t2 + 1) * 512],
                            wqk[c][:, j * 128 : (j + 1) * 128],
                            xr[c][:, t2 * 512 : (t2 + 1) * 512],
                            start=(c == 0),
                            stop=(c == CC - 1),
                        )
                nc.vector.tensor_copy(qkT[j], p)

            # 1b: v[t][:, h*65:h*65+64] = sum_c xr[c][:, t-block].T @ wv[c]
            for t8 in range(NT8):
                # two 384-wide halves at offsets 0 and 512 (a matmul may not
                # cross a PSUM bank boundary)
                p = psum.tile([128, N], F32, tag="big", name="big", bufs=2)
                for c in range(CC):
                    for nh in range(2):
                        nc.tensor.matmul(
                            p[:, nh * 512 : nh * 512 + 384],
                            xr[c][:, t8 * 128 : (t8 + 1) * 128],
                            wv[c][:, nh * 384 : (nh + 1) * 384],
                            start=(c == 0),
                            stop=(c == CC - 1),
                        )
                v_view = v_sb[t8].rearrange("p (h e) -> p h e", e=D + 1)
                for nh in range(2):
                    nc.vector.tensor_copy(
                        v_view[:, nh * 6 : (nh + 1) * 6, 0:D],
                        p[:, nh * 512 : nh * 512 + 384].rearrange(
                            "p (h d) -> p h d", d=D
                        ),
                    )
                nc.vector.tensor_copy(v_view[:, :, D : D + 1], ones12.unsqueeze(2))

            # ---------------- phase 2: attention (SW-pipelined per head) ----
            with tc.tile_pool(name="ph2", bufs=12) as ph2, tc.tile_pool(
                name="ph2s", bufs=4
            ) as ph2s:

                def emit_st_exp(h):
                    kt = qkT[CC + h // 2]
                    qt = qkT[h // 2]
                    po = (h % 2) * 64
                    ex = []
                    for kc in range(NT8):
                        st = psum.tile([128, N], F32, tag="big", name="big", bufs=2)
                        for t2 in range(NT2):
                            nc.tensor.matmul(
                                st[:, t2 * 512 : (t2 + 1) * 512],
                                kt[po : po + D, kc * 128 : (kc + 1) * 128],
                                qt[po : po + D, t2 * 512 : (t2 + 1) * 512],
                                start=True,
                                stop=True,
                            )
                        e = ph2.tile([128, N], F16, tag="exps", name="exps")
                        nc.scalar.activation(
                            e, st, mybir.ActivationFunctionType.Exp, scale=SCALE
                        )
                        ex.append(e)
                    return ex

                def emit_pv_norm(h, ex):
                    po = (h % 2) * 64
                    for t2 in range(NT2):
                        qs = slice(t2 * 512, (t2 + 1) * 512)
                        o = psum.tile([D + 1, 512], F32, tag="ops", name="ops", bufs=2)
                        for kc in range(NT8):
                            nc.tensor.matmul(
                                o,
                                v_sb[kc][:, h * (D + 1) : (h + 1) * (D + 1)],
                                ex[kc][:, qs],
                                start=(kc == 0),
                                stop=(kc == NT8 - 1),
                            )
                        if NORM_DIVIDE:
                            den16 = ph2s.tile([1, 512], F16, tag="den16", name="den16")
                            nc.vector.tensor_copy(den16, o[D : D + 1, :])
                            bc = psum.tile(
                                [D, 512], F32, tag="bcps", name="bcps", bufs=2
                            )
                            nc.tensor.matmul(bc, ones1h, den16, start=True, stop=True)
                            bc_sb = ph2s.tile([D, 512], F32, tag="bcsb", name="bcsb")
                            nc.vector.tensor_copy(bc_sb, bc)
                            nc.vector.tensor_tensor(
                                oT[h // 2][po : po + D, qs],
                                o[0:D, :],
                                bc_sb,
                                mybir.AluOpType.divide,
                            )
                        else:
                            rec = ph2s.tile([1, 512], F32, tag="rec", name="rec")
                            nc.vector.reciprocal(rec, o[D : D + 1, :])
                            rec16 = ph2s.tile([1, 512], F16, tag="rec16", name="rec16")
                            nc.vector.tensor_copy(rec16, rec)
                            bc = psum.tile(
                                [D, 512], F32, tag="bcps", name="bcps", bufs=2
                            )
                            nc.tensor.matmul(bc, ones1h, rec16, start=True, stop=True)
                            bc_sb = ph2s.tile([D, 512], F32, tag="bcsb", name="bcsb")
                            nc.vector.tensor_copy(bc_sb, bc)
                            nc.vector.tensor_mul(
                                oT[h // 2][po : po + D, qs], o[0:D, :], bc_sb
                            )

                prev = None
                for h in range(H):
                    ex = emit_st_exp(h)
                    if prev is not None:
                        emit_pv_norm(h - 1, prev)
                    prev = ex
                emit_pv_norm(H - 1, prev)

            # ---------------- phase 3: output projection ----------------
            with tc.tile_pool(name="ph3o", bufs=3) as ph3o:
                for fc in range(CC):
                    p = psum.tile([128, N], F32, tag="big", name="big", bufs=2)
                    for c in range(CC):
                        for t2 in range(NT2):
                            nc.tensor.matmul(
                                p[:, t2 * 512 : (t2 + 1) * 512],
                                wo[c][:, fc * 128 : (fc + 1) * 128],
                                oT[c][:, t2 * 512 : (t2 + 1) * 512],
                                start=(c == 0),
                                stop=(c == CC - 1),
                            )
                    ot = ph3o.tile([128, N], F32, tag="outsb", name="outsb")
                    nc.vector.tensor_scalar_add(ot, p, bo_t[fc])
                    nc.sync.dma_start(
                        out=outT[fc * 128 : (fc + 1) * 128, :], in_=ot
                    )

    if split:
        _split_multiwaits(nc)
    return nc


_NC = None


def _get_nc():
    global _NC
    if _NC is None:
        _NC = _build()
    return _NC


def kernel(x, w_qkv, w_out, b_out):
    x = np.asarray(x, dtype=np.float32)
    w_qkv = np.asarray(w_qkv, dtype=np.float32)
    w_out = np.asarray(w_out, dtype=np.float32)
    b_out = np.asarray(b_out, dtype=np.float32)

    wqkT = np.ascontiguousarray(w_qkv[: 2 * C].T.astype(np.float16))
    wvT = np.ascontiguousarray(w_qkv[2 * C :].T.astype(np.float16))
    woT = np.ascontiguousarray(w_out.T.astype(np.float16))
    bo = np.ascontiguousarray(b_out.reshape(C, 1))

    in_maps = [
        {
            "xT": np.ascontiguousarray(x[b].T.astype(np.float16)),
            "wqkT": wqkT,
            "wvT": wvT,
            "woT": woT,
            "bo": bo,
        }
        for b in range(B)
    ]

    nc = _get_nc()
    trace = bool(os.environ.get("KERNEL_TRACE"))
    res = run_bass_kernel_spmd(nc, in_maps, list(range(_N_CORES)), trace=trace)
    if trace:
        print(f"HW exec time: {res.exec_time_ns} ns")
        if res.instructions_and_trace is not None:
            print(f"trace: {res.instructions_and_trace[1]}")

    out = np.empty((B, N, C), dtype=np.float32)
    for b in range(B):
        out[b] = res.results[b]["outT"].T
    return out
